# revision 2
# baseline (speedup 1.0000x reference)
"""Trainium2 Bass kernel for a dense transformer block (B=2, T=2048, D=1024, H=16).

Sharding (8 NeuronCores, one chip):
  - Token-parallel LN1/QKV/proj/LN2/MLP: core i owns 256 tokens of batch 0
    (rows 256i:256i+256) and 256 tokens of batch 1 (rows 2048+256i:...).
  - Head-parallel attention: core i owns heads {2i, 2i+1}.
  - Collectives: per-batch AllToAll of (q,k,v) slices to head owners
    ("forward"), per-batch AllToAll of attention outputs back to token
    owners ("backward").  Four collectives total, pipelined so batch-0
    attention overlaps batch-1's forward exchange and batch-0's MLP
    overlaps batch-1's attention/backward exchange.

On-chip activations are feature-major ([feature, token]); LN stats over the
feature axis use ones-vector matmuls on the PE.  Matmul operands are bf16;
accumulation, softmax statistics, LN statistics and residuals stay fp32.
V is shipped token-major with the softmax-denominator ones-columns packed
by the sender, so the receiver's AV matmuls read it directly.
"""

import numpy as np
import ml_dtypes

import concourse.bass as bass
import concourse.mybir as mybir
import concourse.tile as tile
from concourse import bacc
from concourse import bass_utils

F32 = mybir.dt.float32
MM_DT = mybir.dt.bfloat16
MM_NP = ml_dtypes.bfloat16

P = 128            # partitions
TB = 512           # tokens per core (256 per batch)
TH = 256           # tokens per core per batch
D = 1024           # model dim
CT = D // P        # 8 feature tiles
NC = 8             # cores
TOKS = 2 * 2048    # global tokens
T2 = 2048          # tokens per batch
FF = 4096          # mlp hidden
DH = 64            # head dim
VW = 2 * (DH + 1)  # packed V cols per kv tile: 2 heads x (64 + ones) = 130
F8 = mybir.dt.float8e4
CHB = 2 * TH + 4 * VW  # fwd a2a chunk BYTES: q 256 | k 256 | v 520
NJ = 4             # q blocks of 512 per batch
QW = 512           # q block width

AF = mybir.ActivationFunctionType
OP = mybir.AluOpType
RG = [list(range(NC))]

_cache: dict = {}


def _build(spec_ln1=True, spec_ln2b=True):
    nc = bacc.Bacc(
        "TRN2",
        target_bir_lowering=False,
        debug=False,
        enable_asserts=False,
        num_devices=NC,
    )

    # ---- kernel I/O ----
    x_own = nc.dram_tensor("x_own", [TB, D], F32, kind="ExternalInput").ap()
    wqkv = nc.dram_tensor("wqkv", [D, 3 * D], MM_DT, kind="ExternalInput").ap()
    wproj = nc.dram_tensor("wproj", [D, D], MM_DT, kind="ExternalInput").ap()
    wfc1 = nc.dram_tensor("wfc1", [D, FF], MM_DT, kind="ExternalInput").ap()
    wfc2 = nc.dram_tensor("wfc2", [FF, D], MM_DT, kind="ExternalInput").ap()
    id128m = nc.dram_tensor("id128m", [P, P], MM_DT, kind="ExternalInput").ap()
    id128f = nc.dram_tensor("id128f", [P, P], F32, kind="ExternalInput").ap()
    cmask = nc.dram_tensor("cmask", [P, P], F32, kind="ExternalInput").ap()
    if not spec_ln1:
        ln1wb = nc.dram_tensor("ln1wb", [P, D], F32, kind="ExternalInput").ap()
        ln1bb = nc.dram_tensor("ln1bb", [P, D], F32, kind="ExternalInput").ap()
    if not spec_ln2b:
        fc1B = nc.dram_tensor("fc1B", [P, FF // P], F32, kind="ExternalInput").ap()
    out_t = nc.dram_tensor("out_t", [D, TB], F32, kind="ExternalOutput").ap()

    with tile.TileContext(nc) as tc:
        with (
            tc.tile_pool(name="pers", bufs=1) as pers,
            tc.tile_pool(name="dram", bufs=1, space="DRAM") as dram,
        ):
            # ---------- collective buffers ----------
            a2af_in = [dram.tile([NC, P, CHB], F8, name=f"a2af_in{b}",
                                 tag=f"a2af_in{b}") for b in range(2)]
            a2af_out = [dram.tile([NC, P, CHB], F8, name=f"a2af_out{b}",
                                  tag=f"a2af_out{b}") for b in range(2)]
            a2ab_in = [dram.tile([NC, P, TH], MM_DT, name=f"a2ab_in{b}",
                                 tag=f"a2ab_in{b}") for b in range(2)]
            a2ab_out = [dram.tile([NC, P, TH], MM_DT, name=f"a2ab_out{b}",
                                  tag=f"a2ab_out{b}") for b in range(2)]

            # ---------- persistent constants / activations ----------
            ones_col_m = pers.tile([P, 1], MM_DT, name="ones_col_m", tag="ones_col_m")
            nc.gpsimd.memset(ones_col_m[:], 1.0)
            ones_row_m = pers.tile([1, P], MM_DT, name="ones_row_m", tag="ones_row_m")
            nc.gpsimd.memset(ones_row_m[:], 1.0)
            ones_row_f = pers.tile([1, P], F32, name="ones_row_f", tag="ones_row_f")
            nc.gpsimd.memset(ones_row_f[:], 1.0)
            zb = pers.tile([P, 1], F32, name="zb", tag="zb")
            nc.gpsimd.memset(zb[:], 0.0)
            eps1 = pers.tile([1, 1], F32, name="eps1", tag="eps1")
            nc.gpsimd.memset(eps1[:], 1e-5)
            epsP = pers.tile([P, 1], F32, name="epsP", tag="epsP")
            nc.gpsimd.memset(epsP[:], 1e-5)

            idm_sb = pers.tile([P, P], MM_DT, name="idm_sb", tag="idm_sb")
            idf_sb = pers.tile([P, P], F32, name="idf_sb", tag="idf_sb")
            cmask_sb = pers.tile([P, P], F32, name="cmask_sb", tag="cmask_sb")

            xT = [pers.tile([P, TB], F32, name=f"xT{c}", tag=f"xT{c}")
                  for c in range(CT)]
            x2T = [pers.tile([P, TB], F32, name=f"x2T{c}", tag=f"x2T{c}")
                   for c in range(CT)]
            x2m = [pers.tile([P, TB], MM_DT, name=f"x2m{c}", tag=f"x2m{c}")
                   for c in range(CT)]
            x2n = [pers.tile([P, TB], MM_DT, name=f"x2n{c}", tag=f"x2n{c}")
                   for c in range(CT)]
            rs_s = pers.tile([P, TB], F32, name="rs_s", tag="rs_s")
            m_s = pers.tile([P, TB], F32, name="m_s", tag="m_s")
            g1 = [pers.tile([P, TB], MM_DT, name=f"g1_{h}", tag=f"g1_{h}")
                  for h in range(FF // P)]
            wproj_sb = []
            for dtl in range(CT):
                tl = pers.tile([P, D], MM_DT, name=f"wproj_sb{dtl}",
                               tag=f"wproj_sb{dtl}")
                wproj_sb.append(tl)
            if not spec_ln1:
                ln1wb_sb = pers.tile([P, D], F32, name="ln1wb_sb", tag="ln1wb_sb")
                ln1bb_sb = pers.tile([P, D], F32, name="ln1bb_sb", tag="ln1bb_sb")
            if not spec_ln2b:
                fc1B_sb = pers.tile([P, FF // P], F32, name="fc1B_sb",
                                    tag="fc1B_sb")

            # ============ Phase A+B: LN1, QKV, forward AllToAll =============
            with (
                tc.tile_pool(name="work_ab", bufs=1) as work_ab,
                tc.tile_pool(name="ps_ab", bufs=1, space="PSUM") as ps_ab,
            ):
                xrow = [work_ab.tile([P, D], F32, name=f"xrow{r}", tag="xrow",
                                     bufs=4) for r in range(4)]
                xnT = [work_ab.tile([P, TB], MM_DT, name=f"xnT{c}", tag=f"xnT{c}")
                       for c in range(CT)]
                wqkv_sb = []
                # x rows first (they gate the critical path), weights behind
                for r in range(4):
                    nc.sync.dma_start(xrow[r][:], x_own[r * P:(r + 1) * P, :])
                nc.sync.dma_start(idm_sb[:], id128m[:])
                nc.sync.dma_start(idf_sb[:], id128f[:])
                nc.sync.dma_start(cmask_sb[:], cmask[:])
                for c in range(CT):
                    tl = work_ab.tile([P, 3 * D], MM_DT, name=f"wqkv_sb{c}",
                                      tag=f"wqkv_sb{c}")
                    nc.sync.dma_start(tl[:], wqkv[c * P:(c + 1) * P, :])
                    wqkv_sb.append(tl)
                if not spec_ln1:
                    nc.sync.dma_start(ln1wb_sb[:], ln1wb[:])
                    nc.sync.dma_start(ln1bb_sb[:], ln1bb[:])
                for dtl in range(CT):
                    nc.sync.dma_start(wproj_sb[dtl][:],
                                      wproj[dtl * P:(dtl + 1) * P, :])
                if not spec_ln2b:
                    nc.sync.dma_start(fc1B_sb[:], fc1B[:])

                def ln1_tile(r):
                    """Natural-layout LN1 on row-tile r -> bf16 xnT columns.

                    Early tiles (batch 0) do stats on DVE for the fastest
                    start; later tiles use the Act engine to keep DVE free
                    for the apply/copy stream."""
                    s1 = work_ab.tile([P, 1], F32, name=f"s1_{r}", tag="lns1",
                                      bufs=2)
                    s2 = work_ab.tile([P, 1], F32, name=f"s2_{r}", tag="lns2",
                                      bufs=2)
                    if r < 2:
                        nc.vector.tensor_reduce(s1[:], xrow[r][:],
                                                axis=mybir.AxisListType.X,
                                                op=OP.add)
                        sqf = work_ab.tile([P, D], MM_DT, name=f"sqf_{r}",
                                           tag="lnsq", bufs=1)
                        nc.vector.scalar_tensor_tensor(
                            out=sqf[:], in0=xrow[r][:], scalar=1.0,
                            in1=xrow[r][:], op0=OP.mult, op1=OP.mult,
                            accum_out=s2[:])
                    else:
                        dum = work_ab.tile([P, D], MM_DT, name=f"dum_{r}",
                                           tag="lnsq", bufs=1)
                        nc.scalar.activation(dum[:], xrow[r][:], AF.Copy,
                                             accum_out=s1[:])
                        nc.scalar.activation(dum[:], xrow[r][:], AF.Square,
                                             accum_out=s2[:])
                    mu = work_ab.tile([P, 1], F32, name=f"mu_{r}", tag="lnmu",
                                      bufs=2)
                    nc.vector.tensor_scalar_mul(mu[:], s1[:], 1.0 / D)
                    ex2 = work_ab.tile([P, 1], F32, name=f"ex2_{r}", tag="lnex2",
                                       bufs=2)
                    nc.vector.tensor_scalar_mul(ex2[:], s2[:], 1.0 / D)
                    mu2 = work_ab.tile([P, 1], F32, name=f"mu2_{r}", tag="lnmu2",
                                       bufs=2)
                    nc.vector.tensor_mul(mu2[:], mu[:], mu[:])
                    var = work_ab.tile([P, 1], F32, name=f"var_{r}", tag="lnvar",
                                       bufs=2)
                    nc.vector.tensor_sub(var[:], ex2[:], mu2[:])
                    sd = work_ab.tile([P, 1], F32, name=f"sd_{r}",
                                      tag="lnsd", bufs=2)
                    nc.scalar.activation(sd[:], var[:], AF.Sqrt, bias=epsP[:])
                    rstd = work_ab.tile([P, 1], F32, name=f"rstd_{r}",
                                        tag="lnrstd", bufs=2)
                    nc.vector.reciprocal(rstd[:], sd[:])
                    if spec_ln1:
                        xnn = work_ab.tile([P, D], MM_DT, name=f"xnn_{r}",
                                           tag="lnxnn", bufs=2)
                        nc.vector.tensor_scalar(xnn[:], xrow[r][:], mu[:],
                                                rstd[:], OP.subtract, OP.mult)
                    else:
                        xc = work_ab.tile([P, D], F32, name=f"xc_{r}",
                                          tag="lnxc", bufs=2)
                        nc.vector.tensor_scalar(xc[:], xrow[r][:], mu[:],
                                                rstd[:], OP.subtract, OP.mult)
                        xw = work_ab.tile([P, D], F32, name=f"xw_{r}",
                                          tag="lnxw", bufs=2)
                        nc.vector.tensor_mul(xw[:], xc[:], ln1wb_sb[:])
                        xnn = work_ab.tile([P, D], MM_DT, name=f"xnn_{r}",
                                           tag="lnxnn", bufs=2)
                        nc.vector.tensor_add(xnn[:], xw[:], ln1bb_sb[:])
                    for c in range(CT):
                        ptn = ps_ab.tile([P, P], MM_DT, name=f"ptn_{r}_{c}",
                                         tag="ttm", bufs=2)
                        nc.tensor.transpose(ptn[:], xnn[:, c * P:(c + 1) * P],
                                            idm_sb[:])
                        nc.vector.tensor_copy(xnT[c][:, r * P:(r + 1) * P],
                                              ptn[:])

                vstage_t = work_ab.tile([P, 2 * NC * VW], MM_DT,
                                        name="vstage", tag="vstage", bufs=1)
                vstage = [vstage_t, vstage_t]
                qkstage_t = work_ab.tile([P, NC * 2 * TH], F8,
                                         name="qkstage", tag="qkstage", bufs=1)
                qkstage = [qkstage_t, qkstage_t]

                def qkv_batch(b):
                    """Token-major QKV (out = [tokens, qkv features]), then
                    transpose q/k to feature-major for the exchange; v is
                    already token-major and goes straight into the packed
                    V staging with its softmax-ones columns."""
                    nc.gpsimd.memset(vstage[b][:], 1.0)
                    for r in range(2):
                        rcol = slice(b * TH + r * P, b * TH + (r + 1) * P)
                        for p in range(6):
                            ps = ps_ab.tile([P, 4 * P], F32,
                                            name=f"qkv{b}_{r}_{p}",
                                            tag="qkvps", bufs=3)
                            for c in range(CT):
                                nc.tensor.matmul(
                                    ps[:], xnT[c][:, rcol],
                                    wqkv_sb[c][:, p * 512:(p + 1) * 512],
                                    start=(c == 0), stop=(c == CT - 1))
                            tt = work_ab.tile([P, 4 * P], MM_DT,
                                              name=f"tt{b}_{r}_{p}",
                                              tag="qkt", bufs=3)
                            nc.vector.tensor_copy(tt[:], ps[:])
                            if p < 4:  # q (p 0-1) or k (p 2-3): transpose
                                qk_off = 0 if p < 2 else TH
                                for sx in range(4):
                                    j = 4 * (p % 2) + sx
                                    vp = ps_ab.tile([P, P], MM_DT,
                                                    name=f"vp{b}_{r}_{p}_{sx}",
                                                    tag="ttm", bufs=2)
                                    nc.tensor.transpose(
                                        vp[:], tt[:, sx * P:(sx + 1) * P],
                                        idm_sb[:])
                                    dst = j * 2 * TH + qk_off + r * P
                                    nc.vector.tensor_copy(
                                        qkstage[b][:, dst:dst + P], vp[:])
                            else:  # v: token-major, pack with ones columns
                                for sx in range(4):
                                    j = 4 * (p - 4) + sx
                                    base = j * 2 * VW + r * VW
                                    for h in range(2):
                                        nc.vector.tensor_copy(
                                            vstage[b][:, base + h * (DH + 1):
                                                      base + h * (DH + 1) + DH],
                                            tt[:, sx * P + h * DH:
                                               sx * P + (h + 1) * DH])
                    nc.sync.dma_start(
                        a2af_in[b][:, :, 0:2 * TH].transpose([1, 0, 2]),
                        qkstage[b][:])
                    nc.sync.dma_start(
                        a2af_in[b][:, :, 2 * TH:CHB].transpose([1, 0, 2]),
                        vstage[b][:].bitcast(F8))
                    nc.gpsimd.collective_compute(
                        "AllToAll", OP.bypass, replica_groups=RG,
                        ins=[a2af_in[b][:]], outs=[a2af_out[b][:]])

                ln1_tile(0)
                ln1_tile(1)
                qkv_batch(0)
                ln1_tile(2)
                ln1_tile(3)
                qkv_batch(1)
                # raw-x transposes for the residual: fill the collective wait
                for r in range(4):
                    for c in range(CT):
                        pt = ps_ab.tile([P, P], F32, name=f"pt_{r}_{c}",
                                        tag="ttf", bufs=2)
                        nc.tensor.transpose(pt[:], xrow[r][:, c * P:(c + 1) * P],
                                            idf_sb[:])
                        nc.vector.tensor_copy(xT[c][:, r * P:(r + 1) * P], pt[:])

            # ============ Attention (head-parallel, per batch) ==============
            with (
                tc.tile_pool(name="attn_sb", bufs=1) as attn_sb,
                tc.tile_pool(name="ps_attn", bufs=1, space="PSUM") as ps_attn,
            ):
                qT = [attn_sb.tile([P, T2], F8, name=f"qT{b}", tag=f"qT{b}")
                      for b in range(2)]
                kT = [attn_sb.tile([P, T2], F8, name=f"kT{b}", tag=f"kT{b}")
                      for b in range(2)]
                V_sb = [attn_sb.tile([P, 2 * NC * VW], MM_DT, name=f"V_sb{b}",
                                     tag=f"V_sb{b}") for b in range(2)]

                def attn_batch(b):
                    nc.sync.dma_start(
                        kT[b][:],
                        a2af_out[b][:, :, TH:2 * TH].transpose([1, 0, 2]))
                    nc.sync.dma_start(
                        qT[b][:],
                        a2af_out[b][:, :, 0:TH].transpose([1, 0, 2]))
                    nc.sync.dma_start(
                        V_sb[b][:].bitcast(F8),
                        a2af_out[b][:, :, 2 * TH:CHB].transpose([1, 0, 2]))
                    for j4 in range(NJ):
                        psy = [ps_attn.tile([DH + 1, QW], F32,
                                            name=f"psy{b}_{j4}_{h}", tag="psy",
                                            bufs=4) for h in range(2)]
                        nkt = 4 * j4 + 4
                        for kt in range(nkt):
                            d = kt - 4 * j4
                            n0 = d * P if d >= 0 else 0
                            pss = ps_attn.tile([P, 2 * QW], F32,
                                               name=f"pss{b}_{j4}_{kt}",
                                               tag="pss", bufs=2)
                            for h in range(2):
                                nc.tensor.matmul(
                                    pss[:, h * QW + n0:(h + 1) * QW],
                                    kT[b][h * DH:(h + 1) * DH,
                                          kt * P:(kt + 1) * P],
                                    qT[b][h * DH:(h + 1) * DH,
                                          j4 * QW + n0:(j4 + 1) * QW],
                                    start=True, stop=True,
                                    tile_position=(h * DH, 0))
                            if d >= 0:
                                for h in range(2):
                                    nc.vector.tensor_add(
                                        pss[:, h * QW + n0:h * QW + n0 + P],
                                        pss[:, h * QW + n0:h * QW + n0 + P],
                                        cmask_sb[:])
                            et = attn_sb.tile([P, 2 * QW], MM_DT,
                                              name=f"et{b}_{j4}_{kt}",
                                              tag="et", bufs=6)
                            if d >= 0:
                                for h in range(2):
                                    nc.scalar.activation(
                                        et[:, h * QW + n0:(h + 1) * QW],
                                        pss[:, h * QW + n0:(h + 1) * QW],
                                        AF.Exp, bias=zb[:])
                            else:
                                nc.scalar.activation(et[:], pss[:], AF.Exp,
                                                     bias=zb[:])
                            for h in range(2):
                                vsl = V_sb[b][:, kt * VW + h * (DH + 1):
                                              kt * VW + h * (DH + 1) + DH + 1]
                                nc.tensor.matmul(
                                    psy[h][:, n0:QW], vsl,
                                    et[:, h * QW + n0:(h + 1) * QW],
                                    start=(kt == 0), stop=(kt == nkt - 1))
                        # normalize and stage the backward exchange
                        yn = attn_sb.tile([P, QW], MM_DT, name=f"yn{b}_{j4}",
                                          tag="yn", bufs=2)
                        for h in range(2):
                            rf = attn_sb.tile([1, QW], F32,
                                              name=f"rf{b}_{j4}_{h}", tag="rf",
                                              bufs=2)
                            nc.vector.reciprocal(rf[:], psy[h][DH:DH + 1, :])
                            rm = attn_sb.tile([1, QW], MM_DT,
                                              name=f"rm{b}_{j4}_{h}", tag="rm",
                                              bufs=2)
                            nc.vector.tensor_copy(rm[:], rf[:])
                            rb = ps_attn.tile([DH + 1, QW], F32,
                                              name=f"rb{b}_{j4}_{h}",
                                              tag="psy", bufs=4)
                            nc.tensor.matmul(rb[0:DH, :],
                                             ones_row_m[:, 0:DH], rm[:],
                                             start=True, stop=True)
                            rbs = attn_sb.tile([DH, QW], F32,
                                               name=f"rbs{b}_{j4}_{h}",
                                               tag="rbs", bufs=2)
                            nc.vector.tensor_copy(rbs[:], rb[0:DH, :])
                            nc.vector.tensor_mul(
                                yn[h * DH:(h + 1) * DH, :],
                                psy[h][0:DH, :], rbs[:])
                        for sx in range(2):
                            nc.sync.dma_start(
                                a2ab_in[b][2 * j4 + sx],
                                yn[:, sx * TH:(sx + 1) * TH])
                    nc.gpsimd.collective_compute(
                        "AllToAll", OP.bypass, replica_groups=RG,
                        ins=[a2ab_in[b][:]], outs=[a2ab_out[b][:]])

                attn_batch(0)
                attn_batch(1)

            # ============ Tail: batch-fused proj + LN2 + MLP ================
            with (
                tc.tile_pool(name="sb_tail", bufs=1) as sb_tail,
                tc.tile_pool(name="ps_tail", bufs=1, space="PSUM") as ps_tail,
            ):
                _mlp_tail(nc, tc, sb_tail, ps_tail, a2ab_out, wproj_sb,
                          wfc1, wfc2, xT, x2T, x2m, x2n, rs_s, m_s, g1,
                          ones_col_m, ones_row_f, eps1,
                          fc1B_sb if not spec_ln2b else None, out_t)

    nc.compile()
    return nc


def _mlp_tail(nc, tc, sb, ps, a2ab_out, wproj_sb, wfc1, wfc2, xT, x2T, x2m,
              x2n, rs_s, m_s, g1, ones_col_m, ones_row_f, eps1, fc1B_sb,
              out_t):
    """Batch-fused (N=512) proj + residual + LN2 + fc1/gelu + fc2 + output."""
    yall = sb.tile([P, NC * TB], MM_DT, name="yall", tag="yall")
    for b in range(2):
        for dtl in range(NC):
            nc.sync.dma_start(
                yall[:, dtl * TB + b * TH:dtl * TB + (b + 1) * TH],
                a2ab_out[b][dtl])
    small = lambda nm: ps.tile([P, TB], F32, name=nm, tag="small", bufs=2)
    for c in range(CT):
        pp = small(f"proj_{c}")
        for dtl in range(NC):
            nc.tensor.matmul(pp[:],
                             wproj_sb[dtl][:, c * P:(c + 1) * P],
                             yall[:, dtl * TB:(dtl + 1) * TB],
                             start=(dtl == 0), stop=(dtl == NC - 1))
        nc.vector.tensor_add(x2T[c][:], pp[:], xT[c][:])
        nc.vector.tensor_copy(x2m[c][:], x2T[c][:])
    # LN2 stats over features via ones-matmuls (separate banks for s1/s2)
    s1t = small("s1t")
    s2t = small("s2t")
    s1, s2 = s1t[0:1, :], s2t[0:1, :]
    for c in range(CT):
        nc.tensor.matmul(s1, ones_col_m[:], x2m[c][:],
                         start=(c == 0), stop=(c == CT - 1))
        sq = sb.tile([P, TB], MM_DT, name=f"sq_{c}", tag="sq", bufs=2)
        nc.vector.tensor_mul(sq[:], x2m[c][:], x2m[c][:])
        nc.tensor.matmul(s2, ones_col_m[:], sq[:],
                         start=(c == 0), stop=(c == CT - 1))
    mu = sb.tile([1, TB], F32, name="mu2t", tag="lnmu")
    nc.vector.tensor_scalar_mul(mu[:], s1, 1.0 / D)
    mu2 = sb.tile([1, TB], F32, name="mu2sq", tag="lnmu2")
    nc.vector.tensor_mul(mu2[:], mu[:], mu[:])
    var = sb.tile([1, TB], F32, name="var2", tag="lnvar")
    nc.vector.scalar_tensor_tensor(
        out=var[:], in0=s2, scalar=1.0 / D, in1=mu2[:],
        op0=OP.mult, op1=OP.subtract)
    sd = sb.tile([1, TB], F32, name="sd2", tag="lnsd")
    nc.scalar.activation(sd[:], var[:], AF.Sqrt, bias=eps1[:])
    rs_row = sb.tile([1, TB], F32, name="rs_row", tag="lnrs")
    nc.vector.reciprocal(rs_row[:], sd[:])
    mrow = sb.tile([1, TB], F32, name="mrow", tag="lnmr")
    nc.vector.tensor_mul(mrow[:], mu[:], rs_row[:])
    bc1 = small("bc1")
    nc.tensor.matmul(bc1[:], ones_row_f[:], rs_row[:], start=True, stop=True)
    nc.vector.tensor_copy(rs_s[:], bc1[:])
    bc2 = small("bc2")
    nc.tensor.matmul(bc2[:], ones_row_f[:], mrow[:], start=True, stop=True)
    nc.vector.tensor_copy(m_s[:], bc2[:])
    # normalize once: x2n = x2*rstd - mu*rstd (bf16)
    for c in range(CT):
        eng = nc.vector if c % 2 == 0 else nc.gpsimd
        t1 = sb.tile([P, TB], F32, name=f"t1_{c}", tag="t1", bufs=4)
        eng.tensor_mul(t1[:], x2T[c][:], rs_s[:])
        eng.tensor_sub(x2n[c][:], t1[:], m_s[:])
    # fc1 + gelu, with fc2 pass 1 (output chunks 0..3) interleaved per ht
    HB = 8
    ps_m1 = [ps.tile([P, TB], F32, name=f"fc2a_{i}", tag="fc2A", bufs=4)
             for i in range(4)]
    for blk in range(FF // (HB * P)):
        w1t = []
        for c in range(CT):
            wt = sb.tile([P, HB * P], MM_DT, name=f"w1t{blk}_{c}",
                         tag=f"w1t{c}", bufs=2)
            nc.sync.dma_start(
                wt[:], wfc1[c * P:(c + 1) * P,
                            blk * HB * P:(blk + 1) * HB * P])
            w1t.append(wt)
        for hh in range(HB):
            ht = blk * HB + hh
            pf = small(f"fc1_{ht}")
            for c in range(CT):
                nc.tensor.matmul(pf[:],
                                 w1t[c][:, hh * P:(hh + 1) * P],
                                 x2n[c][:],
                                 start=(c == 0), stop=(c == CT - 1))
            bias = fc1B_sb[:, ht:ht + 1] if fc1B_sb is not None else 0.0
            nc.scalar.activation(g1[ht][:], pf[:], AF.Gelu, bias=bias)
            w2t = sb.tile([P, D // 2], MM_DT, name=f"w2ta_{ht}", tag="w2ta",
                          bufs=4)
            nc.sync.dma_start(w2t[:], wfc2[ht * P:(ht + 1) * P, 0:D // 2])
            for i in range(4):
                nc.tensor.matmul(ps_m1[i][:], w2t[:, i * P:(i + 1) * P],
                                 g1[ht][:],
                                 start=(ht == 0), stop=(ht == FF // P - 1))
    for i in range(4):
        x3 = sb.tile([P, TB], F32, name=f"x3a_{i}", tag="x3", bufs=2)
        nc.vector.tensor_add(x3[:], ps_m1[i][:], x2T[i][:])
        nc.sync.dma_start(out_t[i * P:(i + 1) * P, :], x3[:])
    # fc2 pass 2 (output chunks 4..7)
    ps_m2 = [ps.tile([P, TB], F32, name=f"fc2b_{i}", tag="fc2A", bufs=4)
             for i in range(4)]
    for ht in range(FF // P):
        w2t = sb.tile([P, D // 2], MM_DT, name=f"w2tb_{ht}", tag="w2ta",
                      bufs=4)
        nc.sync.dma_start(w2t[:], wfc2[ht * P:(ht + 1) * P, D // 2:D])
        for i in range(4):
            nc.tensor.matmul(ps_m2[i][:], w2t[:, i * P:(i + 1) * P],
                             g1[ht][:],
                             start=(ht == 0), stop=(ht == FF // P - 1))
    for i in range(4):
        c = 4 + i
        x3 = sb.tile([P, TB], F32, name=f"x3b_{i}", tag="x3", bufs=2)
        nc.vector.tensor_add(x3[:], ps_m2[i][:], x2T[c][:])
        nc.sync.dma_start(out_t[c * P:(c + 1) * P, :], x3[:])


def _prep_inputs(x, ln1_w, ln1_b, w_qkv, w_proj, ln2_w, ln2_b, w_fc1, w_fc2):
    xf = np.ascontiguousarray(np.asarray(x, np.float32).reshape(TOKS, D))
    wq = np.asarray(w_qkv[:, :D], np.float32) * 0.125  # fold 1/sqrt(dh)
    wk = np.asarray(w_qkv[:, D:2 * D], np.float32)
    wv = np.asarray(w_qkv[:, 2 * D:], np.float32)
    wqkv_full = np.ascontiguousarray(
        np.concatenate([wq, wk, wv], axis=1)).astype(MM_NP)
    wproj = np.asarray(w_proj, np.float32).astype(MM_NP)
    wfc2 = np.asarray(w_fc2, np.float32).astype(MM_NP)
    idm = np.eye(P, dtype=np.float32).astype(MM_NP)
    idf = np.eye(P, dtype=np.float32)
    pp, jj = np.meshgrid(np.arange(P), np.arange(P), indexing="ij")
    cm = np.where(pp <= jj, 0.0, -1e30).astype(np.float32)
    # fold ln2_w into fc1
    w1p = (np.asarray(ln2_w, np.float32)[:, None] * np.asarray(w_fc1, np.float32))
    wfc1 = w1p.astype(MM_NP)
    spec_ln1 = bool(np.allclose(np.asarray(ln1_w, np.float32), 1.0) and
                    np.allclose(np.asarray(ln1_b, np.float32), 0.0))
    spec_ln2b = bool(np.allclose(np.asarray(ln2_b, np.float32), 0.0))
    common = {
        "wqkv": wqkv_full, "wproj": wproj, "wfc1": wfc1, "wfc2": wfc2,
        "id128m": idm, "id128f": idf, "cmask": cm,
    }
    if not spec_ln1:
        ln1w_f = np.asarray(ln1_w, np.float32)
        ln1b_f = np.asarray(ln1_b, np.float32)
        common["ln1wb"] = np.ascontiguousarray(np.broadcast_to(ln1w_f, (P, D)))
        common["ln1bb"] = np.ascontiguousarray(np.broadcast_to(ln1b_f, (P, D)))
    if not spec_ln2b:
        Bv = np.asarray(ln2_b, np.float32) @ np.asarray(w_fc1, np.float32)
        common["fc1B"] = np.ascontiguousarray(Bv.reshape(FF // P, P).T)
    in_maps = []
    for i in range(NC):
        m = dict(common)
        m["x_own"] = np.ascontiguousarray(np.concatenate(
            [xf[TH * i:TH * (i + 1)],
             xf[T2 + TH * i:T2 + TH * (i + 1)]], axis=0))
        in_maps.append(m)
    _cache["spec"] = (spec_ln1, spec_ln2b)
    return in_maps


def _get_runner():
    spec_ln1, spec_ln2b = _cache.get("spec", (True, True))
    key = ("runner", spec_ln1, spec_ln2b)
    if key in _cache:
        return _cache[key]
    import jax
    from jax.sharding import Mesh, PartitionSpec, NamedSharding
    from jax.experimental.shard_map import shard_map
    from concourse import bass2jax

    nc = _cache.get("nc")
    if nc is None or _cache.get("nc_spec") != (spec_ln1, spec_ln2b):
        nc = _cache["nc"] = _build(spec_ln1, spec_ln2b)
        _cache["nc_spec"] = (spec_ln1, spec_ln2b)
    bass2jax.install_neuronx_cc_hook()
    partition_name = nc.partition_id_tensor.name if nc.partition_id_tensor else None
    in_names, out_names, out_avals, zero_outs = [], [], [], []
    for alloc in nc.m.functions[0].allocations:
        if not isinstance(alloc, mybir.MemoryLocationSet):
            continue
        name = alloc.memorylocations[0].name
        if alloc.kind == "ExternalInput":
            if name != partition_name:
                in_names.append(name)
        elif alloc.kind == "ExternalOutput":
            out_names.append(name)
            shape = tuple(alloc.tensor_shape)
            dtype = mybir.dt.np(alloc.dtype)
            out_avals.append(jax.core.ShapedArray(shape, dtype))
            zero_outs.append(np.zeros(shape, dtype))
    n_params = len(in_names)
    all_in_names = in_names + out_names + ([partition_name] if partition_name else [])

    def _body(*args):
        operands = list(args)
        if partition_name is not None:
            operands.append(bass2jax.partition_id_tensor())
        outs = bass2jax._bass_exec_p.bind(
            *operands, out_avals=tuple(out_avals), in_names=tuple(all_in_names),
            out_names=tuple(out_names), lowering_input_output_aliases=(),
            sim_require_finite=True, sim_require_nnan=True, nc=nc)
        return tuple(outs)

    devices = jax.devices()[:NC]
    mesh = Mesh(np.asarray(devices), ("core",))
    nin = n_params + len(out_names)
    sharded = jax.jit(shard_map(
        _body, mesh=mesh, in_specs=(PartitionSpec("core"),) * nin,
        out_specs=(PartitionSpec("core"),) * len(out_names), check_rep=False))
    sh = NamedSharding(mesh, PartitionSpec("core"))
    dev_zeros = [
        jax.device_put(np.zeros((NC * z.shape[0], *z.shape[1:]), z.dtype), sh)
        for z in zero_outs
    ]
    runner = (sharded, in_names, out_names, out_avals, sh, dev_zeros)
    _cache[key] = runner
    return runner


def kernel(**inputs):
    import jax
    in_maps = _prep_inputs(**inputs)
    sharded, in_names, out_names, out_avals, sh, dev_zeros = _get_runner()
    concat_in = [np.concatenate([in_maps[c][nm] for c in range(NC)], axis=0)
                 for nm in in_names]
    dev_in = [jax.device_put(a, sh) for a in concat_in]
    out_arrs = sharded(*dev_in, *dev_zeros)
    got = {nm: np.asarray(out_arrs[i]).reshape(NC, *out_avals[i].shape)
           for i, nm in enumerate(out_names)}
    out = np.empty((TOKS, D), np.float32)
    for i in range(NC):
        blk = got["out_t"][i].T  # [TB, D]
        out[TH * i:TH * (i + 1)] = blk[0:TH]
        out[T2 + TH * i:T2 + TH * (i + 1)] = blk[TH:TB]
    return out.reshape(2, 2048, D)


if __name__ == "__main__":
    rng = np.random.default_rng(0)
    ins = {
        "x": rng.standard_normal((2, 2048, D), dtype=np.float32),
        "ln1_w": np.ones(D, np.float32),
        "ln1_b": np.zeros(D, np.float32),
        "w_qkv": (rng.standard_normal((D, 3 * D), dtype=np.float32) / 32.0),
        "w_proj": (rng.standard_normal((D, D), dtype=np.float32) / 32.0),
        "ln2_w": np.ones(D, np.float32),
        "ln2_b": np.zeros(D, np.float32),
        "w_fc1": (rng.standard_normal((D, FF), dtype=np.float32) / 32.0),
        "w_fc2": (rng.standard_normal((FF, D), dtype=np.float32) / 64.0),
    }
    out = kernel(**ins)
    print("kernel out", out.shape, out.dtype, float(np.abs(out).mean()))
